# revision 25
# baseline (speedup 1.0000x reference)
"""Multi-head attention + output projection, sharded over 8 NeuronCores.

Shapes: Q/K/V [2, 2048, 1024], mask [1,1,2048,2048] (zeros), W [1024,1024],
b [1024]. The reference does a *direct* reshape (B, H, S, Dh) of (B, S, D),
which means head h of batch b is rows [128h, 128h+128) of Q[b] reinterpreted
as a contiguous (2048, 64) block.  The 32 (b, h) pairs are data-parallel:
core c owns pairs 4c..4c+3 and also computes the output projection for the
rows of x those pairs produce, so no collectives are needed.

Per-core kernel (inputs bf16, accumulation fp32). The softmax exp of
16.8M score elements per core is the roofline; it is SPLIT between the
ACT engine (real exp, ~1.11us per [128,1024] fp32 PSUM tile) and the
DVE (Schraudolph int16 bit-trick exp2, ~1.22us/tile; 1x rate since the
PSUM source is fp32 — TRN2 matmuls cannot emit bf16 to PSUM):
  S^T[j, q] = sum_d K[j,d] Q[q,d]          (row-packed pairs of K=64 matmuls)
  P^T = exp(S^T / 8)  (ACT: Exp with scale folded in; DVE: tensor_scalar
                       affine + fp32->int16 floor + bf16 bitcast; scores
                       ~N(0,1)*8 so no max-subtraction needed)
  Otil^T[0:64]   = V^T @ P^T               (accumulating mm family, bf16)
  Otil^T[64:128] = colsums(P^T)            (64 ones-columns appended to V,
                                            i.e. sums arrive pre-broadcast)
  O'^T = Otil^T[0:64] / Otil^T[64:128]     (DVE per-ql chain: copy sums to
         SBUF + reciprocal_approx_fast + multiply; [64,512]-shaped ops —
         wider/taller merged variants of this chain return garbage on HW)
  x^T  = layout shuffle of O'^T (SBUF->SBUF DMA; queries processed in a
         host-permuted cb-major order to make the shuffle contiguous)
  y    = x @ W^T + b  (W^T bf16 / fp32 bias fed by host; projection emitted
         as per-ql quarters so only 4 matmuls trail the last softmax)

PSUM budget (8 banks x 2KB): score ring 2x[128,1024]f32 = 4 banks, PV
accumulator [128,1024]f32 (both ql chunks of a q-half) = 2 banks,
projection accumulator [128,1024]f32 = 2 banks.

Measured-on-HW dead ends, kept for the record: bf16 matmul output to
PSUM (TRN3-only, would enable DVE 2x exp); merged [128,1024] norm chain
(garbage on HW, fine in CoreSim); a 3-deep score ring with ql-serial
windows + 1-bank PV/projection accumulators (166-172us: more window
boundaries -> more cross-engine stalls); staging softmax sums through
ScalarE copy (NaNs); reciprocal_approx_fast reading PSUM directly
(garbage on HW); mid-run shuffles on the GPSIMD SWDGE queue (slower).
"""

import math

import numpy as np

B, S, DMODEL, HEADS = 2, 2048, 1024, 16
DH = DMODEL // HEADS  # 64
N_CORES = 8
PAIRS = 4  # (b, h) pairs per core
ROWS = PAIRS * 128  # x/y rows per core (512)

# Engine assignment for the 16 exp tiles of each q-half (index =
# kbp*2 + ql): 'A' = ACT real exp, 'D' = DVE Schraudolph exp2.
# 10A/6D per q-half -> 80 ACT / 48 DVE tiles per core.
EXP_PATTERN = "AADAADADAADAADAD"

_CACHE = {}


def _build_nc():
    import concourse.mybir as mybir
    import concourse.tile as tile
    from concourse import bacc
    from concourse.bass import ds, ts

    f32 = mybir.dt.float32
    bf16 = mybir.dt.bfloat16
    i16 = mybir.dt.int16
    Exp = mybir.ActivationFunctionType.Exp
    Mult = mybir.AluOpType.mult
    Add = mybir.AluOpType.add

    # Schraudolph exp2 bit trick for the DVE-assigned tiles:
    # exp(s/8) ~= bf16_bits( int16( s * (0.125*log2e*128) + (127-sigma)*128 ) )
    SCH_A = 0.125 * 1.4426950408889634 * 128.0
    SCH_B = (127.0 - 0.0445) * 128.0

    nc = bacc.Bacc(None, target_bir_lowering=False)

    QKV = nc.declare_dram_parameter("QKV", [PAIRS, 128, 6144], bf16, isOutput=False)
    WB = nc.declare_dram_parameter("WB", [128, 10240], bf16, isOutput=False)
    OUT = nc.declare_dram_parameter("OUT", [ROWS, DMODEL], f32, isOutput=True)

    with tile.TileContext(nc) as tc:
        with (
            tc.tile_pool(name="const", bufs=1) as constp,
            tc.tile_pool(name="work", bufs=3) as workp,
            tc.tile_pool(name="norm", bufs=2) as normp,
            tc.tile_pool(name="pt", bufs=8) as ptp,
            tc.tile_pool(name="psS", bufs=2, space="PSUM") as psS,
            tc.tile_pool(name="psP", bufs=1, space="PSUM") as psP,
            tc.tile_pool(name="psO", bufs=2, space="PSUM") as psO,
        ):
            wb_sb = constp.tile([128, 10240], bf16, tag="wb")
            wt_sb = wb_sb[:, 0:8192].rearrange("p (mc o) -> p mc o", mc=8, o=1024)
            b_sb = wb_sb[:, 8192:10240].bitcast(f32)
            # Pair 0's QKT-gating chunks go FIRST — before even the table
            # warm — so the transfers that gate the first exp start at t=0.
            qkv0 = workp.tile([128, 6144], bf16, tag="qkv", name="qkv0")
            nc.sync.dma_start(qkv0[:, 2048:2304], QKV[0][:, 2048:2304])
            nc.scalar.dma_start(qkv0[:, 0:512], QKV[0][:, 0:512])
            # Warm the ACT exp table during the first input DMA.
            warm = constp.tile([1, 64], f32, tag="warm")
            nc.vector.memset(warm[:], 0.5)
            nc.scalar.activation(warm[:], warm[:], Exp)

            def issue_qkv(p):
                if p == 0:
                    qkv = qkv0
                    nc.sync.dma_start(qkv[:, 2304:2816], QKV[p][:, 2304:2816])
                    nc.sync.dma_start(qkv[:, 512:1024], QKV[p][:, 512:1024])
                    nc.sync.dma_start(qkv[:, 2816:3072], QKV[p][:, 2816:3072])
                    nc.sync.dma_start(qkv[:, 4096:4608], QKV[p][:, 4096:4608])
                    nc.sync.dma_start(qkv[:, 4608:5120], QKV[p][:, 4608:5120])
                else:
                    qkv = workp.tile([128, 6144], bf16, tag="qkv", name=f"qkv{p}")
                    nc.sync.dma_start(qkv[:, 2048:3072], QKV[p][:, 2048:3072])
                    nc.sync.dma_start(qkv[:, 0:1024], QKV[p][:, 0:1024])
                    nc.sync.dma_start(qkv[:, 4096:4608], QKV[p][:, 4096:4608])
                nc.sync.dma_start(qkv[:, 3072:4096], QKV[p][:, 3072:4096])
                if p > 0:
                    nc.sync.dma_start(qkv[:, 4608:5120], QKV[p][:, 4608:5120])
                nc.sync.dma_start(qkv[:, 1024:2048], QKV[p][:, 1024:2048])
                nc.sync.dma_start(qkv[:, 5120:6144], QKV[p][:, 5120:6144])
                return qkv

            qkv_pre = [issue_qkv(0), issue_qkv(1)]
            for wc in range(0, 10240, 2048):
                nc.sync.dma_start(wb_sb[:, wc : wc + 2048], WB[:, wc : wc + 2048])

            for p in range(PAIRS):
                qkv = qkv_pre[p] if p < 2 else issue_qkv(p)
                qt = qkv[:, 0:2048]
                kt = qkv[:, 2048:4096]
                vt = qkv[:, 4096:6144].rearrange("p (kb v) -> p kb v", kb=16, v=128)

                xts = workp.tile([128, 8, 128], bf16, tag="xts")
                py = psP.tile([128, 1024], f32, tag="py")

                for qh in range(2):
                    # Per-ql single-bank PV accumulators from a 2-deep pool:
                    # each ql's bank is freed by its own norm chain, so the
                    # next q-half's PV for that ql never waits on the other
                    # ql's normalization.
                    po = [
                        psO.tile([128, 512], f32, tag="po", name=f"po{i}")
                        for i in range(2)
                    ]
                    for kbp in range(8):
                        kbA, kbB = 2 * kbp, 2 * kbp + 1
                        ptb = ptp.tile([128, 2, 1024], bf16, tag="pt")
                        for ql in range(2):
                            qq = 2 * qh + ql
                            ps = psS.tile([128, 1024], f32, tag="ps")
                            with tc.high_priority(offset=40):
                                nc.tensor.matmul(
                                    ps[:, 0:512],
                                    kt[0:64][:, ts(kbA, 128)],
                                    qt[0:64][:, ts(qq, 512)],
                                    start=True,
                                    stop=True,
                                )
                                nc.tensor.matmul(
                                    ps[:, 512:1024],
                                    kt[64:128][:, ts(kbB, 128)],
                                    qt[64:128][:, ts(qq, 512)],
                                    start=True,
                                    stop=True,
                                )
                            if EXP_PATTERN[kbp * 2 + ql] == "D":
                                nc.vector.tensor_scalar(
                                    ptb[:, ql].bitcast(i16),
                                    ps[:],
                                    SCH_A,
                                    SCH_B,
                                    Mult,
                                    Add,
                                )
                            else:
                                nc.scalar.activation(
                                    ptb[:, ql], ps[:], Exp, scale=1.0 / math.sqrt(DH)
                                )
                        for slot, kb in ((0, kbA), (1, kbB)):
                            for ql in range(2):
                                nc.tensor.matmul(
                                    po[ql][:],
                                    vt[:, kb, :],
                                    ptb[:, ql, ds(slot * 512, 512)],
                                    start=(kb == 0),
                                    stop=(kb == 15),
                                )

                    # Normalize each ql chunk: O'^T = Otil^T[0:64] / sums.
                    # Boosted so the DVE drains the PV accumulator before
                    # starting the next q-half's exp tiles — the next
                    # q-half's first PV matmul waits on exactly this chain.
                    at_tail = p == PAIRS - 1 and qh == 1
                    osc2 = []
                    for ql in range(2):
                        bcs = normp.tile([64, 512], f32, tag="bcs")
                        nc.vector.tensor_copy(bcs[:], po[ql][64:128, :])
                        bcr = normp.tile([64, 512], f32, tag="bcr")
                        nc.vector.reciprocal_approx_fast(bcr[:], bcs[:])
                        o = normp.tile([64, 512], bf16, tag="osc")
                        nc.vector.tensor_mul(o[:], po[ql][0:64, :], bcr[:])
                        osc2.append(o)

                    for ql in range(2):
                        osc = osc2[ql]
                        srcv = osc[:, 0:512].rearrange(
                            "d (c1 c0 r) -> d c0 c1 r", c1=2, c0=2, r=128
                        )
                        for c0 in range(2):
                            dmae = (
                                nc.scalar
                                if (at_tail and (ql + c0) % 2 == 1)
                                else nc.sync
                            )
                            dmae.dma_start(
                                xts[ds(c0 * 64, 64), ds(4 * qh + 2 * ql, 2), :],
                                srcv[:, c0],
                            )
                        if at_tail and ql == 0:
                            pk = psS.tile([128, 1024], f32, tag="ps")
                            nc.tensor.matmul(
                                pk[:, 0:512],
                                osc[:, 0:128],
                                osc[:, 0:512],
                                start=True,
                                stop=True,
                            )
                        with tc.high_priority(offset=0 if at_tail else -100):
                            for oh in range(2):
                                for mc in (4 * qh + 2 * ql, 4 * qh + 2 * ql + 1):
                                    nc.tensor.matmul(
                                        py[:, ds(oh * 512, 512)],
                                        xts[:, mc, :],
                                        wt_sb[:, mc, ds(oh * 512, 512)],
                                        start=(mc == 0),
                                        stop=(mc == 7),
                                    )

                yt = workp.tile([128, 1024], f32, tag="yt")
                for oh in range(2):
                    nc.vector.tensor_add(
                        yt[:, ds(oh * 512, 512)],
                        py[:, ds(oh * 512, 512)],
                        b_sb[:, ds(oh * 512, 512)],
                    )
                    dmae = nc.scalar if (p == PAIRS - 1 and oh == 1) else nc.sync
                    dmae.dma_start(
                        OUT[ts(p, 128), ds(oh * 512, 512)], yt[:, ds(oh * 512, 512)]
                    )

    nc.finalize()
    return nc


def _host_prep(Q, K, V, W, b):
    """Build the 8 per-core input maps (host-side shard + transpose + bf16)."""
    import ml_dtypes

    bf16 = np.dtype(ml_dtypes.bfloat16)

    Q = np.ascontiguousarray(Q, dtype=np.float32)
    K = np.ascontiguousarray(K, dtype=np.float32)
    V = np.ascontiguousarray(V, dtype=np.float32)
    W = np.ascontiguousarray(W, dtype=np.float32)
    b = np.ascontiguousarray(b, dtype=np.float32)

    WBh = np.empty((128, 10240), dtype=bf16)
    WBh[:, 0:8192] = (
        W.T.reshape(8, 128, DMODEL).transpose(1, 0, 2).reshape(128, 8192)
    ).astype(bf16)
    bias_bits = (
        np.broadcast_to(b[None, :], (128, DMODEL))
        .astype(np.float32)
        .copy()
        .view(np.uint16)
    )
    WBh.view(np.uint16)[:, 8192:10240] = bias_bits

    in_maps = []
    for c in range(N_CORES):
        QKVh = np.empty((PAIRS, 128, 6144), dtype=bf16)
        QT2 = QKVh[:, :, 0:2048]
        KT2 = QKVh[:, :, 2048:4096]
        Vth = QKVh[:, :, 4096:6144].reshape(PAIRS, 128, 16, 2 * DH)
        for pl in range(PAIRS):
            pair = 4 * c + pl
            bb, h = pair // HEADS, pair % HEADS
            Qh = Q[bb, 128 * h : 128 * (h + 1), :].reshape(S, DH)
            Kh = K[bb, 128 * h : 128 * (h + 1), :].reshape(S, DH)
            Vh = V[bb, 128 * h : 128 * (h + 1), :].reshape(S, DH)
            QhTp = (
                Qh.T.reshape(DH, 128, 16).transpose(0, 2, 1).reshape(DH, S)
            ).astype(bf16)
            QT2[pl, 0:64] = QhTp
            QT2[pl, 64:128] = QT2[pl, 0:64]
            KT2[pl, 0:64] = Kh.T.astype(bf16)
            KT2[pl, 64:128] = KT2[pl, 0:64]
            Vth[pl, :, :, 0:DH] = (
                Vh.reshape(16, 128, DH).transpose(1, 0, 2).astype(bf16)
            )
            Vth[pl, :, :, DH : 2 * DH] = 1.0
        in_maps.append({"QKV": QKVh, "WB": WBh})
    return in_maps


def _gather(results):
    y = np.empty((B, S, DMODEL), dtype=np.float32)
    for c in range(N_CORES):
        out_c = results[c]["OUT"]
        for pl in range(PAIRS):
            pair = 4 * c + pl
            bb, h = pair // HEADS, pair % HEADS
            y[bb, 128 * h : 128 * (h + 1), :] = out_c[128 * pl : 128 * (pl + 1), :]
    return y


def _run(inputs, trace=False, **kw):
    from concourse.bass_utils import run_bass_kernel_spmd

    if "nc" not in _CACHE:
        _CACHE["nc"] = _build_nc()
    nc = _CACHE["nc"]
    in_maps = _host_prep(
        inputs["Q"], inputs["K"], inputs["V"], inputs["W"], inputs["b"]
    )
    res = run_bass_kernel_spmd(nc, in_maps, list(range(N_CORES)), trace=trace, **kw)
    return _gather(res.results), res


def _numpy_fallback(Q, K, V, mask, W, b):
    q = Q.reshape(B, HEADS, S, DH)
    k = K.reshape(B, HEADS, S, DH)
    v = V.reshape(B, HEADS, S, DH)
    scale = 1.0 / math.sqrt(DH)
    out = np.empty((B, HEADS, S, DH), dtype=np.float32)
    m = np.asarray(mask, dtype=np.float32)[0, 0]
    for bb in range(B):
        for h in range(HEADS):
            s = q[bb, h].astype(np.float64) @ k[bb, h].astype(np.float64).T * scale
            s = s + m
            s -= s.max(axis=1, keepdims=True)
            e = np.exp(s)
            p = e / e.sum(axis=1, keepdims=True)
            out[bb, h] = p @ v[bb, h].astype(np.float64)
    x = out.reshape(B, S, DMODEL)
    return (x @ W.T + b).astype(np.float32)


def kernel(Q, K, V, mask, W, b):
    Q, K, V, mask, W, b = (np.asarray(t) for t in (Q, K, V, mask, W, b))
    if np.any(mask):
        return _numpy_fallback(Q, K, V, mask, W, b)
    y, _ = _run({"Q": Q, "K": K, "V": V, "W": W, "b": b})
    return y
